# revision 1
# baseline (speedup 1.0000x reference)
"""NT-Xent loss kernel for 8 Trainium2 NeuronCores (Bass/Tile).

Symmetric data-parallel strategy (each unordered pair computed once):
  - host: z = concat(z_i, z_j) [16384, 256] f32. Core c receives z rotated by
    its row offset: rot_c[i] = z[(2048c + i) % 16384]. With that rotation the
    IR is identical across cores: local rows are rot rows [0, 2048) and the
    core's column window is rot rows [0, 8192) - each unordered pair {i, j}
    lands in exactly one core's (local rows x window) block (pairs at offset
    exactly 8192 - the positive pairs - are excluded and handled on host).
  - device (identical SPMD IR on all 8 cores), ACT-exp-saturated schedule:
      * all input DMA is issued up front, split across the Sync and GpSimd
        hardware DMA queues; the first window group loads quarter-granular
        so the normalize chain (fused square-dot on DVE, rnorm = exp(-ln/2)
        on ACT - same table set as the main Exp) starts as data trickles in.
      * window rows are normalized (bf16) and PE-transposed into per-group
        zn^T fp8 tiles. The next group's norm chain runs early in the
        current group's program order and its transposes are interleaved
        into cg1's PE stream, so ACT never waits at group boundaries.
        zn^T group 0 doubles as the local-row lhsT.
      * main loop over 8 x 1024-col PSUM groups x 16 local row tiles:
        2 accumulated fp8 DoubleRow matmuls (K=256) -> one ACT Exp per tile
        (bf16 es, fused fp32 row-sum via accum_out) -> two ones-matmuls
        accumulate the block's column sums in PSUM, emitted with a 3-row
        lag so the in-order PE queue never blocks on the ACT semaphore.
      * per-row diagonal dot (fp8, matches the PE diagonal) and fp32
        positive-pair dot as fused DVE dots in otherwise-idle windows.
  - host (fp64): expsum[i] = own rowsum + the 4 covering cores' colsums
    + exp(10*pos_i) - exp(10*diag_i) - bf16(exp(10*diag_i));
    loss = mean(log(expsum) - 10*pos).
"""

import os
import numpy as np

try:
    import concourse.bass as bass
except ImportError:  # pragma: no cover
    import sys

    sys.path.insert(0, "/opt/trn_rl_repo")
    import concourse.bass as bass

import concourse.mybir as mybir
import concourse.tile as tile
from concourse.bass_utils import run_bass_kernel_spmd

F32 = mybir.dt.float32
BF16 = mybir.dt.bfloat16
FP8 = mybir.dt.float8e4

B = 8192
D = 256
N = 2 * B  # 16384
NCORES = 8
RPC = N // NCORES  # 2048 local rows per core
RT = RPC // 128  # 16 local row tiles
HT = RT // 2  # tiles per norm half
W = N // 2  # 8192-column window per core
GB = 4  # window load/transpose groups (16 tiles each)
GW = W // GB  # 2048 columns of znT per group tile
CG = 8  # main-loop column groups
CW = W // CG  # 1024 cols per PSUM group (2 banks)
SUB = 512  # matmul free dim (1 PSUM bank)
TEMP_INV = 10.0  # 1 / temperature
EPS2 = 1e-16  # cos eps^2 (clamp on squared norm)
LAG = 3  # colsum row lag (PE never waits on ACT)
YIELD_R = 10  # cg_loop mid-point for prefetch emission

# set by the last run when BASS_TRACE=1 (read by test.py)
last_exec_time_ns = None
last_mean_exec_time_ns = None

_CACHE = {}


def _fixup_bir(bir_bytes):
    """Adapt Tile-emitted BIR to this container's walrus build:
    - split instructions carrying >1 sync wait (walrus allows one per inst)
    - replace the raw-ISA EVENT_SEMAPHORE_RANGE_CLEAR (encoding mismatch)
      with per-semaphore sem-wr-imm zero writes."""
    import json

    b = json.loads(bir_bytes)
    for fn in b["functions"]:
        for blk in fn["blocks"]:
            new_ins = []
            for ins in blk["instructions"]:
                if (
                    ins.get("opcode") == "ISA"
                    and ins.get("op_name") == "EVENT_SEMAPHORE_RANGE_CLEAR"
                ):
                    d = ins["ant_dict"]
                    for s in range(d["range_first"], d["range_last"] + 1):
                        new_ins.append(
                            {
                                "debug": ins.get("debug", 0),
                                "engine": ins["engine"],
                                "ins": [],
                                "outs": [],
                                "name": f'{ins["name"]}_z{s}',
                                "opcode": "EventSemaphore",
                                "sync_info": {
                                    "on_update": [
                                        {
                                            "ant_name": f"zero_{s}",
                                            "id": s,
                                            "sync_type": "semaphore",
                                            "update_mode": "sem-wr-imm",
                                            "update_value": 0,
                                        }
                                    ],
                                    "on_wait": [],
                                },
                            }
                        )
                    continue
                si = ins.get("sync_info")
                if si:
                    waits = si.get("on_wait") or []
                    if len(waits) > 1:
                        for j, w in enumerate(waits[:-1]):
                            new_ins.append(
                                {
                                    "debug": ins.get("debug", 0),
                                    "engine": ins["engine"],
                                    "ins": [],
                                    "outs": [],
                                    "name": f'{ins["name"]}_w{j}',
                                    "opcode": "EventSemaphore",
                                    "sync_info": {"on_update": [], "on_wait": [w]},
                                }
                            )
                        si["on_wait"] = [waits[-1]]
                new_ins.append(ins)
            blk["instructions"] = new_ins
    return json.dumps(b).encode()


_PATCHED = False


def _install_bir_fixup():
    """Route the pjrt compile path's BIR bytes through _fixup_bir."""
    global _PATCHED
    if _PATCHED:
        return
    from concourse import bass2jax

    orig = bass2jax._decompress_ant_bir

    def patched(ant_bir_value):
        return _fixup_bir(orig(ant_bir_value))

    bass2jax._decompress_ant_bir = patched
    _PATCHED = True


def _emit(tc, nc, z_win, z_pos, ident_in, out, out_c):
    from contextlib import ExitStack

    Exp = mybir.ActivationFunctionType.Exp
    Ln = mybir.ActivationFunctionType.Ln
    Copy = mybir.ActivationFunctionType.Copy
    DR = mybir.MatmulPerfMode.DoubleRow
    X = mybir.AxisListType.X
    MUL = mybir.AluOpType.mult
    BYP = mybir.AluOpType.bypass

    with ExitStack() as ctx:
        singles = ctx.enter_context(tc.tile_pool(name="singles", bufs=1))
        zbig = ctx.enter_context(tc.tile_pool(name="zbig", bufs=8))
        znb = ctx.enter_context(tc.tile_pool(name="znb", bufs=34))
        dmp = ctx.enter_context(tc.tile_pool(name="dmp", bufs=2))
        st = ctx.enter_context(tc.tile_pool(name="st", bufs=8))
        esp = ctx.enter_context(tc.tile_pool(name="es", bufs=8))
        # 2-deep ps (4 banks) + colsum accumulator (2) + transpose staging
        # (2) = all 8 PSUM banks. A 3-deep ps was tried and is a wash: the
        # deeper PE run-ahead slows ACT's PSUM reads ~90ns/instr via write
        # port contention, cancelling the idle it removes.
        mmp = ctx.enter_context(tc.tile_pool(name="mmp", bufs=2, space="PSUM"))
        csp = ctx.enter_context(tc.tile_pool(name="csp", bufs=1, space="PSUM"))
        tpp = ctx.enter_context(tc.tile_pool(name="tpp", bufs=2, space="PSUM"))

        ident = singles.tile([128, 128], BF16)
        ones = singles.tile([128, 128], BF16)
        nc.vector.memset(ones, 1.0)

        znT = [
            singles.tile([128, 2, GW], FP8, name=f"znT_{i}", tag=f"znT_{i}")
            for i in range(GB)
        ]
        diag = singles.tile([128, RT], F32)
        posd = singles.tile([128, RT], F32)
        rawp = singles.tile([128, RT], F32)
        rnl = singles.tile([128, RT], F32)
        ss_p = singles.tile([128, RT], F32)
        racc = singles.tile([128, RT, CG], F32)
        csb = singles.tile([1, CG, CW], F32)
        zf8 = singles.tile([128, RT, D], FP8)
        zp = singles.tile([128, RT, D], F32)
        o_sb = singles.tile([128, RT, 4], F32)
        nc.vector.memset(o_sb, 0.0)

        def fdot(a, b, accum_col):
            """accum_col [128,1] = sum(a*b) along free dim, one fused DVE op."""
            dump = dmp.tile([128, D], F32, name="dump", tag="dump")
            nc.vector.scalar_tensor_tensor(
                out=dump, in0=a, scalar=1.0, in1=b, op0=BYP, op1=MUL,
                accum_out=accum_col,
            )

        def rnorm(ss, tag):
            """ss [128, k] squared norms -> 1/max(sqrt(ss), eps) as
            exp(-0.5*ln(ss)); Ln+Exp share the main Exp's ACT table set.
            high_priority: the moment the dots land these must preempt the
            queued exp stream, not sit behind it."""
            ln = st.tile(list(ss.shape), F32, name="ln", tag=tag + "_ln")
            rn = st.tile(list(ss.shape), F32, name="rn", tag=tag + "_rn")
            # no eps clamp: ||z||^2 ~ D >> eps^2 for any non-degenerate
            # input, so max(ss, EPS2) == ss exactly; dropping it removes a
            # DVE hop + semaphore from every rnorm bubble in the exp stream
            with tc.high_priority():
                nc.scalar.activation(out=ln, in_=ss, func=Ln)
                nc.scalar.activation(out=rn, in_=ln, func=Exp, scale=-0.5)
            return rn

        def zb_dma(gb, h, eng, quarters=1):
            """One window half [128, HT, D] on the given DMA queue."""
            zb = zbig.tile([128, HT, D], F32, name="zb", tag="zb")
            r0 = gb * GW + h * HT * 128
            step = HT // quarters
            for q in range(quarters):
                eng.dma_start(
                    out=zb[:, q * step : (q + 1) * step, :],
                    in_=z_win[
                        r0 + q * step * 128 : r0 + (q + 1) * step * 128, :
                    ].rearrange("(k p) d -> p k d", p=128),
                )
            return zb

        def zb_dots(zbs, tag):
            """Fused norm dots for both halves -> [(zb, ssf), ...]."""
            halves = []
            for zb in zbs:
                ssf = st.tile([128, HT], F32, name="ssf", tag=tag + "_ssf")
                for t in range(HT):
                    fdot(zb[:, t, :], zb[:, t, :], ssf[:, t : t + 1])
                halves.append((zb, ssf))
            return halves

        def scale_tile(zb, j, rn, on_act=False):
            """znf tile = zb[:, j, :] * rn[:, j] (bf16); optionally on the
            ACT engine (Copy with scale) when it is idle at startup."""
            zt = znb.tile([128, D], BF16, name="zt", tag="znf")
            if on_act:
                nc.scalar.activation(
                    out=zt, in_=zb[:, j, :], func=Copy, scale=rn[:, j : j + 1]
                )
            else:
                nc.vector.tensor_scalar_mul(
                    out=zt, in0=zb[:, j, :], scalar1=rn[:, j : j + 1]
                )
            return zt

        def stage2(halves, tag):
            """rnorms first (ACT unblocked), then bf16 scale-casts split
            across DVE and GPSIMD."""
            rns = [rnorm(ssf, f"{tag}{h}") for h, (_, ssf) in enumerate(halves)]
            znf = []
            for h, (zb, _) in enumerate(halves):
                for j in range(HT):
                    znf.append(scale_tile(zb, j, rns[h]))
            return znf, rns

        def fullchunk(gb, znf, idx, cast_on_act=False, split_cast=False):
            """8 PE transposes + fp8 cast: znT[gb][:, dhalf, h*1024+].
            The cast runs on DVE, or on the (startup-idle) ACT engine;
            split_cast emits it as two half-width copies so the first main
            matmul's columns become ready before the later transposes."""
            h, dhalf = idx // 2, idx % 2
            tp = tpp.tile([128, 1024], BF16, name="tp", tag="tp")
            for j in range(8):
                t = h * 8 + j
                nc.tensor.transpose(
                    tp[:, j * 128 : (j + 1) * 128],
                    znf[t][:, dhalf * 128 : (dhalf + 1) * 128],
                    ident,
                )
            c0 = h * 1024
            parts = ((0, 512), (512, 1024)) if split_cast else ((0, 1024),)
            for a, b in parts:
                dst = znT[gb][:, dhalf, c0 + a : c0 + b]
                if cast_on_act:
                    nc.scalar.activation(out=dst, in_=tp[:, a:b], func=Copy)
                else:
                    # high_priority: frees the tp PSUM slot ASAP so the
                    # next chunk's transposes never block the PE queue
                    with tc.high_priority():
                        nc.vector.tensor_copy(dst, tp[:, a:b])

        # ---- main column-group loop: generator yields once after YIELD_R
        # so the caller can emit prefetch work at that program point ------
        def cg_loop(gb, cg, tr, lag=LAG):
            off = (cg * CW) % GW
            cs = csp.tile([128, CW], F32, name="cs", tag="cs")
            es_tiles = []

            def colsum(k):
                for s in range(CW // SUB):
                    nc.tensor.matmul(
                        cs[:, s * SUB : (s + 1) * SUB],
                        lhsT=ones,
                        rhs=es_tiles[k][:, s * SUB : (s + 1) * SUB],
                        start=(k == 0),
                        stop=(k == RT - 1),
                    )

            for r in range(RT):
                if tr is not None and r in (2, 6, 10, 14):
                    # wait hint: keep the scheduler from hoisting this
                    # chunk's transpose burst ahead of its r-slot (modeled
                    # time; deliberately ~12% low so it can't arrive late)
                    slot_ms = 0.012 + (cg * 16 + r) * 0.00105
                    with tc.tile_wait_until(slot_ms):
                        fullchunk(tr[0], tr[1], (r - 2) // 4)
                ps = mmp.tile([128, CW], F32, name="ps", tag="ps")
                lhsT = znT[0][:, :, r * 128 : (r + 1) * 128]
                for s in range(CW // SUB):
                    c0 = off + s * SUB
                    nc.tensor.matmul(
                        ps[:, s * SUB : (s + 1) * SUB],
                        lhsT=lhsT,
                        rhs=znT[gb][:, :, c0 : c0 + SUB],
                        start=True,
                        stop=True,
                        perf_mode=DR,
                    )
                if r >= lag:
                    colsum(r - lag)
                es = esp.tile([128, CW], BF16, name="es", tag="es")
                es_tiles.append(es)
                nc.scalar.activation(
                    out=es,
                    in_=ps,
                    func=Exp,
                    scale=TEMP_INV,
                    accum_out=racc[:, r, cg : cg + 1],
                )
                if r == YIELD_R:
                    yield
            for k in range(RT - lag, RT):
                colsum(k)
            # colsum rows are identical (ones lhsT); stream the group's
            # slice to DRAM as soon as it's extracted (overlapped DMA).
            # high_priority: the cs PSUM bank gates the next group's first
            # colsum accumulation.
            with tc.high_priority():
                nc.vector.tensor_copy(csb[0:1, cg, :], cs[0:1, :])
            nc.sync.dma_start(out=out_c[cg : cg + 1, :], in_=csb[0:1, cg, :])

        def run_cg(gen):
            for _ in gen:
                pass

        # ================= prologue ====================================
        # gb0 h0 quarter-granular on both DMA queues; dots trickle in.
        # Queue order tuned so the h0 quarters land first (ident is only
        # needed by the first transposes, ~8us later).
        nc.sync.dma_start(out=ident, in_=ident_in[:, :])
        zb00 = zbig.tile([128, HT, D], F32, name="zb", tag="zb")
        for q, eng in ((0, nc.sync), (1, nc.gpsimd), (3, nc.sync), (2, nc.gpsimd)):
            r0 = q * (HT // 4) * 128
            eng.dma_start(
                out=zb00[:, q * 2 : q * 2 + 2, :],
                in_=z_win[r0 : r0 + 256, :].rearrange("(k p) d -> p k d", p=128),
            )
        zb01 = zb_dma(0, 1, nc.gpsimd, quarters=2)
        ssf00 = st.tile([128, HT], F32, name="ssf", tag="g0_ssf")
        for t in range(HT):
            fdot(zb00[:, t, :], zb00[:, t, :], ssf00[:, t : t + 1])
        rn00 = rnorm(ssf00, "g0h0")
        znf0 = []
        for j in range(HT):
            znf0.append(scale_tile(zb00, j, rn00, on_act=(j % 2 == 1)))
        fullchunk(0, znf0, 0, cast_on_act=True)
        fullchunk(0, znf0, 1)
        # h1 chain + remaining chunks (wait hint: the scheduler's DMA model
        # is optimistic; keep these from jumping ahead of the h0 chain)
        ssf01 = st.tile([128, HT], F32, name="ssf", tag="g0_ssf1")
        with tc.tile_wait_until(0.013):
            for t in range(HT):
                fdot(zb01[:, t, :], zb01[:, t, :], ssf01[:, t : t + 1])
        rn01 = rnorm(ssf01, "g0h1")
        for j in range(HT):
            znf0.append(scale_tile(zb01, j, rn01))
        fullchunk(0, znf0, 2, cast_on_act=True)
        fullchunk(0, znf0, 3)
        nc.vector.tensor_copy(rnl[:, 0:HT], rn00)
        nc.vector.tensor_copy(rnl[:, HT:RT], rn01)
        # remaining input DMA, deadline-ordered, alternating queues
        zb_rest = {}
        for g in (1, 2, 3):
            zb_rest[g] = [zb_dma(g, 0, nc.sync), zb_dma(g, 1, nc.gpsimd)]
        nc.sync.dma_start(
            out=zp[:, 0:HT, :],
            in_=z_pos[0 : HT * 128, :].rearrange("(k p) d -> p k d", p=128),
        )
        nc.gpsimd.dma_start(
            out=zp[:, HT:RT, :],
            in_=z_pos[HT * 128 : RPC, :].rearrange("(k p) d -> p k d", p=128),
        )
        with tc.tile_wait_until(0.028):
            halves_next = zb_dots(zb_rest[1], "g1")

        # ================= main loop ===================================
        znf_next = None
        rn_p = None
        for gb in range(GB):
            g = cg_loop(gb, 2 * gb, None)
            if gb == 1:
                # fp8 copies of the gb0 local zn rows for the PE-parity
                # diag dot (before the znf pool slots rotate to gb2)
                with tc.tile_wait_until(0.058):
                    for t in range(RT):
                        nc.vector.tensor_copy(zf8[:, t, :], znf0[t])
            if gb == 2:
                with tc.tile_wait_until(0.095):
                    for t in range(RT):
                        fdot(zf8[:, t, :], zf8[:, t, :], diag[:, t : t + 1])
            next(g)  # emits r=0..YIELD_R
            if gb + 1 < GB:
                znf_next, _ = stage2(halves_next, f"g{gb + 1}_")
            if gb == 1:
                # positive-pair raw dots (gb0 zbig tiles stay allocated)
                with tc.tile_wait_until(0.065):
                    for t in range(RT):
                        h, j = t // HT, t % HT
                        src = zb00 if h == 0 else zb01
                        fdot(src[:, j, :], zp[:, t, :], rawp[:, t : t + 1])
            if gb == 2:
                rn_p = rnorm(ss_p, "p")
            run_cg(g)

            tr = (gb + 1, znf_next) if gb + 1 < GB else None
            # last group: shorter colsum lag so the final drain chain after
            # the last exp is one colsum instead of three
            g = cg_loop(gb, 2 * gb + 1, tr, lag=1 if gb == GB - 1 else LAG)
            next(g)
            if gb == 1:
                with tc.tile_wait_until(0.075):
                    for t in range(RT):
                        fdot(zp[:, t, :], zp[:, t, :], ss_p[:, t : t + 1])
            if gb == 2:
                nc.vector.tensor_mul(posd, rawp, rnl)
                nc.vector.tensor_mul(posd, posd, rn_p)
            if gb == 3:
                nc.vector.tensor_copy(o_sb[:, :, 1:2], diag)
                nc.vector.tensor_copy(o_sb[:, :, 2:3], posd)
            run_cg(g)
            if gb + 2 < GB:
                with tc.tile_wait_until(0.045 + 0.037 * gb):
                    halves_next = zb_dots(zb_rest[gb + 2], f"g{gb + 2}")

        # ---- finalize (out is p-major [128, RT, 4]: contiguous DMA, on
        # the gpsimd queue so it flows parallel to the last colsum slice)
        nc.vector.reduce_sum(out=o_sb[:, :, 0:1], in_=racc, axis=X)
        nc.gpsimd.dma_start(out=out[:, :, :], in_=o_sb)


def build_program():
    if "nc" in _CACHE:
        return _CACHE["nc"]
    nc = bass.Bass()
    z_win = nc.declare_dram_parameter("z_win", [W, D], F32, isOutput=False)
    z_pos = nc.declare_dram_parameter("z_pos", [RPC, D], F32, isOutput=False)
    ident = nc.declare_dram_parameter("ident", [128, 128], BF16, isOutput=False)
    out = nc.declare_dram_parameter("out", [128, RT, 4], F32, isOutput=True)
    out_c = nc.declare_dram_parameter("out_c", [CG, CW], F32, isOutput=True)
    with tile.TileContext(nc) as tc:
        _emit(
            tc, nc, z_win[:, :], z_pos[:, :], ident[:, :], out[:, :, :], out_c[:, :]
        )
    _CACHE["nc"] = nc
    return nc


def make_in_maps(z):
    import ml_dtypes

    eye = np.eye(128, dtype=ml_dtypes.bfloat16)
    zz = np.concatenate([z, z], axis=0)  # easy wraparound slicing
    in_maps = []
    for c in range(NCORES):
        r0 = c * RPC
        p0 = (r0 + B) % N
        in_maps.append(
            {
                "z_win": np.ascontiguousarray(zz[r0 : r0 + W]),
                "z_pos": zz[p0 : p0 + RPC],
                "ident": eye,
            }
        )
    return in_maps


def finalize(row_outs, col_outs):
    """row_outs: per-core [RPC, 4]; col_outs: per-core [CG, CW] -> loss.

    The rowsum (fp32 accum, pre-cast) includes the diagonal as
    exp(10*diag); the colsum (bf16 es summed on the PE) includes it as
    bf16(exp(10*diag)). Subtract both."""
    import ml_dtypes

    # row_outs are p-major [128, RT, 4]: row (k*128 + p) lives at [p, k]
    row_outs = [r.transpose(1, 0, 2).reshape(RPC, 4) for r in row_outs]
    o = np.concatenate(row_outs, axis=0).astype(np.float64)  # [N, 4]
    rowsum, diagd, posd = o[:, 0], o[:, 1], o[:, 2]
    expsum = rowsum.copy()
    for c in range(NCORES):
        idx = (c * RPC + np.arange(W)) % N
        np.add.at(expsum, idx, col_outs[c].reshape(-1).astype(np.float64))
    ediag = np.exp(TEMP_INV * diagd)
    ediag_bf16 = ediag.astype(np.float32).astype(ml_dtypes.bfloat16).astype(np.float64)
    expsum += np.exp(TEMP_INV * posd) - ediag - ediag_bf16
    lse = np.log(expsum)
    return np.float32(np.mean(lse - TEMP_INV * posd))


def _enable_axon_trace_hook():
    """Best-effort: register the NTFF profile hook that the image's antenv
    stub does not ship, and neuter the artifact upload (no bucket creds
    in this container). Only needed when profiling (BASS_TRACE=1)."""
    import sys
    import types

    try:
        from antenv import axon_hooks  # noqa: F401
    except ImportError:
        try:
            import antenv
            from trn_agent_boot.trn_boot import _ntff_profile_via_ctypes

            mod = types.ModuleType("antenv.axon_hooks")
            _hook = [None]
            mod.set_axon_ntff_profile_hook = lambda h: _hook.__setitem__(0, h)
            mod.get_axon_ntff_profile_hook = lambda: _hook[0]
            sys.modules["antenv.axon_hooks"] = mod
            antenv.axon_hooks = mod
            mod.set_axon_ntff_profile_hook(
                _ntff_profile_via_ctypes("/opt/axon/libaxon_pjrt.so")
            )
        except Exception as e:  # pragma: no cover
            print(f"trace hook setup failed: {e}")
    try:
        from concourse import bass_utils as _bu

        _bu.upload_artifacts = lambda tmpdir: f"local:{tmpdir}"
    except Exception:
        pass


def kernel(z_i, z_j, logit_scale_m=None, **_unused):
    global last_exec_time_ns, last_mean_exec_time_ns
    z_i = np.ascontiguousarray(np.asarray(z_i, dtype=np.float32))
    z_j = np.ascontiguousarray(np.asarray(z_j, dtype=np.float32))
    assert z_i.shape == (B, D) and z_j.shape == (B, D)
    z = np.concatenate([z_i, z_j], axis=0)

    nc = build_program()
    in_maps = make_in_maps(z)
    _install_bir_fixup()
    trace = bool(os.environ.get("BASS_TRACE"))
    if trace:
        _enable_axon_trace_hook()
    res = run_bass_kernel_spmd(nc, in_maps, list(range(NCORES)), trace=trace)
    last_exec_time_ns = res.exec_time_ns
    last_mean_exec_time_ns = res.mean_exec_time_ns
    row_outs = [res.results[c]["out"] for c in range(NCORES)]
    col_outs = [res.results[c]["out_c"] for c in range(NCORES)]
    return np.asarray(finalize(row_outs, col_outs), dtype=np.float32)

